# revision 43
# baseline (speedup 1.0000x reference)
"""Bahdanau additive attention on 8 Trainium2 NeuronCores.

Data-parallel: batch dim B=8 sharded 1 batch per core; weights replicated.

Per-core math (Te=256 enc steps, Td=128 dec steps, H=512):
  enc_w = enc_out @ W            [Te, H]
  dec_u = dec_out @ U            [Td, H]
  scores[d,t] = sum_k v[k] * tanh(enc_w[t,k] + dec_u[d,k])
  e = softmax(scores, axis=t)    [Td, Te]
  c = e @ enc_out                [Td, H]

Layout: k (=H) lives on SBUF partitions (4 chunks of 128). The add
broadcasts dec_u^T[:, d] as a per-partition scalar on DVE; tanh runs as
large-free-dim ACT instructions; the v-weighted k-reduction runs on PE
with a replicated-column stationary (v (x) ones_32) col-tiled so that
decoder step d = 32*j + g lands on PSUM partition d directly.
"""

import os
import numpy as np

B, Te, Td, H = 8, 256, 128, 512
N_CORES = 8
KC = H // 128  # 4 k-chunks

# score-path dtype: "f32" or "bf16"
SCORE_DTYPE = os.environ.get("BAHDANAU_SCORE_DTYPE", "f32")

_BUILT = {}


def _build():
    key = SCORE_DTYPE
    if key in _BUILT:
        return _BUILT[key]

    import concourse.bass as bass
    import concourse.tile as tile
    from concourse import bacc, mybir

    dt = mybir.dt
    f32 = dt.float32
    f32r = dt.float32r
    # score-path storage dtype for the broadcast-add sums (ACT tanh input).
    # f32r = fp32 bits, PE processes at 1 cycle/row (vs 4 for strict fp32).
    sdt = dt.float32r if SCORE_DTYPE == "f32" else dt.bfloat16
    # tanh OUTPUT / v-reduce dtype: walrus rejects f32r matmuls with
    # tile_position (col-tiling), so the v-weighted reduce runs in fp16
    # (tanh outputs live in [-1,1] -> fp16's 10-bit mantissa beats bf16 4x);
    # the tanh args themselves stay exact fp32.
    tdt = dt.float16

    nc = bacc.Bacc("TRN2", target_bir_lowering=False, debug=False)

    enc_d = nc.dram_tensor("enc", [Te, H], f32r, kind="ExternalInput")
    encT_d = nc.dram_tensor("encT", [H, Te], f32r, kind="ExternalInput")
    decT_d = nc.dram_tensor("decT", [H, Td], f32r, kind="ExternalInput")
    W_d = nc.dram_tensor("W", [H, H], f32r, kind="ExternalInput")
    U_d = nc.dram_tensor("U", [H, H], f32r, kind="ExternalInput")
    vrep_d = nc.dram_tensor("vrep", [H, 32], tdt, kind="ExternalInput")
    masks_d = nc.dram_tensor("masks", [32, 128, Te], dt.uint8, kind="ExternalInput")
    ident_d = nc.dram_tensor("ident", [128, 128], f32, kind="ExternalInput")
    c_d = nc.dram_tensor("c_out", [Td, H], f32, kind="ExternalOutput")
    e_d = nc.dram_tensor("e_out", [Td, Te], f32, kind="ExternalOutput")

    X = mybir.AxisListType.X
    Tanh = mybir.ActivationFunctionType.Tanh
    Exp = mybir.ActivationFunctionType.Exp

    with tile.TileContext(nc) as tc:
        with (
            tc.tile_pool(name="const", bufs=1) as const,
            tc.tile_pool(name="ew", bufs=1) as ew,
            tc.tile_pool(name="sums", bufs=4) as sums,
            tc.tile_pool(name="tanhp", bufs=4) as tanhp,
            tc.tile_pool(name="sm", bufs=1) as sm,
            tc.tile_pool(name="vred", bufs=4, space=bass.MemorySpace.PSUM) as vred,
            tc.tile_pool(name="misc_ps", bufs=2, space=bass.MemorySpace.PSUM) as misc_ps,
        ):
            # ---- load inputs ----
            # W/U are loaded per 128-column block so the first enc_wT /
            # dec_uT matmuls unblock as early as possible.
            encT_sb = const.tile([128, KC, Te], f32r, tag="encT_sb")
            nc.sync.dma_start(encT_sb[:], encT_d.ap().rearrange("(c p) t -> p c t", p=128))
            decT_sb = const.tile([128, KC, Td], f32r, tag="decT_sb")
            nc.sync.dma_start(decT_sb[:], decT_d.ap().rearrange("(c p) d -> p c d", p=128))
            W_sb = const.tile([128, KC, H], f32r, tag="W_sb")
            W_re = W_d.ap().rearrange("(c p) k -> p c k", p=128)
            U_sb = const.tile([128, KC, H], f32r, tag="U_sb")
            U_re = U_d.ap().rearrange("(c p) k -> p c k", p=128)
            for m in range(KC):
                nc.sync.dma_start(
                    W_sb[:, :, m * 128:(m + 1) * 128], W_re[:, :, m * 128:(m + 1) * 128]
                )
                nc.sync.dma_start(
                    U_sb[:, :, m * 128:(m + 1) * 128], U_re[:, :, m * 128:(m + 1) * 128]
                )
            vrep_sb = const.tile([128, KC, 32], tdt, tag="vrep_sb")
            nc.sync.dma_start(vrep_sb[:], vrep_d.ap().rearrange("(c p) m -> p c m", p=128))
            masks_sb = const.tile([128, 32, Te], dt.uint8, tag="masks_sb")
            nc.sync.dma_start(masks_sb[:], masks_d.ap().rearrange("g p t -> p g t"))

            # ---- enc_wT[k, t] = sum_h' W[h', k] * encT[h', t]  (4 m-chunks),
            # interleaved with dec_uT so chunk m of both is ready early ----
            enc_wT = ew.tile([128, KC, Te], sdt, tag="enc_wT")
            dec_uT = ew.tile([128, KC, Td], f32, tag="dec_uT")
            for m in range(KC):
                ps = misc_ps.tile([128, H], f32, tag="misc")
                for kk in range(KC):
                    nc.tensor.matmul(
                        ps[:, :Te],
                        W_sb[:, kk, m * 128:(m + 1) * 128],
                        encT_sb[:, kk, :],
                        start=(kk == 0),
                        stop=(kk == KC - 1),
                    )
                nc.scalar.copy(enc_wT[:, m, :], ps[:, :Te])
                ps = misc_ps.tile([128, H], f32, tag="misc")
                for kk in range(KC):
                    nc.tensor.matmul(
                        ps[:, :Td],
                        U_sb[:, kk, m * 128:(m + 1) * 128],
                        decT_sb[:, kk, :],
                        start=(kk == 0),
                        stop=(kk == KC - 1),
                    )
                nc.scalar.copy(dec_uT[:, m, :], ps[:, :Td])

            # ---- scores via per-quad pipeline ----
            # quad g handles decoder steps {g, 32+g, 64+g, 96+g}
            scores = sm.tile([128, Te], f32, tag="scores")
            for g in range(32):
                S = sums.tile([128, 16, Te], sdt, tag="S")
                for kc in range(KC):
                    for j in range(4):
                        d = 32 * j + g
                        nc.vector.tensor_scalar_add(
                            S[:, kc * 4 + j, :],
                            enc_wT[:, kc, :],
                            dec_uT[:, kc, d:d + 1],
                        )
                T = tanhp.tile([128, 16, Te], tdt, tag="T")
                edge = g < 2 or g == 31
                if edge:
                    # fine-grained tanh at the edges: at the start ACT can
                    # begin once enc_wT/dec_uT chunk 0 exists; at the end the
                    # PE reduce overlaps the tanh tail instead of serializing
                    for kc in range(KC):
                        s0 = kc * 4
                        nc.scalar.activation(
                            T[:, s0:s0 + 4, :], S[:, s0:s0 + 4, :], Tanh
                        )
                else:
                    nc.scalar.activation(T[:], S[:], Tanh)
                P = vred.tile([128, Te], f32, tag="P")
                # for the last quad run the accumulation kc-major so the PE
                # starts on chunk 0 while chunk 1..3 tanh still runs (the four
                # per-col-group accumulation chains interleave; has_written
                # is per element so this is safe on HW)
                loop = (
                    [(j, kc) for kc in range(KC) for j in range(4)]
                    if g == 31
                    else [(j, kc) for j in range(4) for kc in range(KC)]
                )
                for j, kc in loop:
                    nc.tensor.matmul(
                        P[32 * j:32 * (j + 1), :],
                        vrep_sb[:, kc, :],
                        T[:, kc * 4 + j, :],
                        start=(kc == 0),
                        stop=(kc == KC - 1),
                        tile_position=(0, 32 * j),
                        skip_group_check=(g == 31),
                    )
                nc.vector.copy_predicated(scores[:], masks_sb[:, g, :], P[:])

            # inputs needed only by the tail, loaded at low priority
            enc_sb = const.tile([128, 2, H], f32r, tag="enc_sb")
            nc.sync.dma_start(enc_sb[:], enc_d.ap().rearrange("(c p) h -> p c h", p=128))
            ident_sb = const.tile([128, 128], f32, tag="ident_sb")
            nc.sync.dma_start(ident_sb[:], ident_d.ap())

            # ---- softmax over t (free dim) + context c = e @ enc_out ----
            # context uses the UNNORMALIZED exp u and folds 1/Z into the
            # final PSUM->SBUF copy, so the e^T transpose and context matmul
            # don't wait on the row-sum/reciprocal.
            u = sm.tile([128, Te], f32, tag="u")
            nc.scalar.activation(u[:], scores[:], Exp)
            uT = sm.tile([128, 2, 128], f32r, tag="uT")
            for tc_i in range(2):
                ps = misc_ps.tile([128, H], f32, tag="misc")
                nc.tensor.transpose(
                    ps[:, :128], u[:, tc_i * 128:(tc_i + 1) * 128], ident_sb[:]
                )
                nc.scalar.copy(uT[:, tc_i, :], ps[:, :128])
            z = sm.tile([128, 1], f32, tag="z")
            nc.vector.tensor_reduce(z[:], u[:], axis=X, op=mybir.AluOpType.add)
            rz = sm.tile([128, 1], f32, tag="rz")
            nc.vector.reciprocal(rz[:], z[:])
            e_sb = sm.tile([128, Te], f32, tag="e_sb")
            nc.vector.tensor_scalar_mul(e_sb[:], u[:], rz[:, 0:1])
            nc.sync.dma_start(e_d.ap(), e_sb[:])
            cps = misc_ps.tile([128, H], f32, tag="misc")
            for tc_i in range(2):
                nc.tensor.matmul(
                    cps[:],
                    uT[:, tc_i, :],
                    enc_sb[:, tc_i, :],
                    start=(tc_i == 0),
                    stop=(tc_i == 1),
                )
            c_sb = sm.tile([128, H], f32, tag="c_sb")
            for h in range(2):
                nc.scalar.activation(
                    c_sb[:, h * 256:(h + 1) * 256],
                    cps[:, h * 256:(h + 1) * 256],
                    mybir.ActivationFunctionType.Copy,
                    scale=rz[:, 0:1],
                )
                nc.sync.dma_start(
                    c_d.ap()[:, h * 256:(h + 1) * 256], c_sb[:, h * 256:(h + 1) * 256]
                )

    nc.compile()
    _BUILT[key] = nc
    return nc


def _make_in_maps(enc_out, dec_out, W, U, v):
    sdt_np = np.float16
    enc_out = np.asarray(enc_out, dtype=np.float32)
    dec_out = np.asarray(dec_out, dtype=np.float32)
    W = np.ascontiguousarray(np.asarray(W, dtype=np.float32))
    U = np.ascontiguousarray(np.asarray(U, dtype=np.float32))
    v = np.asarray(v, dtype=np.float32)
    vrep = np.ascontiguousarray(np.repeat(v.reshape(H, 1), 32, axis=1)).astype(sdt_np)
    ident = np.eye(128, dtype=np.float32)
    masks = np.zeros((32, 128, Te), np.uint8)
    for g in range(32):
        masks[g, g::32, :] = 1
    in_maps = []
    for b in range(B):
        in_maps.append(
            {
                "enc": np.ascontiguousarray(enc_out[b]),
                "encT": np.ascontiguousarray(enc_out[b].T),
                "decT": np.ascontiguousarray(dec_out[b].T),
                "W": W,
                "U": U,
                "vrep": vrep,
                "masks": masks,
                "ident": ident,
            }
        )
    return in_maps


def run(enc_out, dec_out, W, U, v, trace=False):
    """Build+run on 8 cores. Returns (c, e, BassKernelResults)."""
    from concourse.bass_utils import run_bass_kernel_spmd

    nc = _build()
    in_maps = _make_in_maps(enc_out, dec_out, W, U, v)
    res = run_bass_kernel_spmd(nc, in_maps, list(range(N_CORES)), trace=trace)
    rs = res.results
    c = np.stack([np.asarray(rs[i]["c_out"]) for i in range(N_CORES)])
    e = np.stack([np.asarray(rs[i]["e_out"]) for i in range(N_CORES)])
    return c.astype(np.float32), e.astype(np.float32), res


def kernel(enc_out, dec_out, W, U, v):
    c, e, _ = run(enc_out, dec_out, W, U, v)
    return c, e


# revision 44
# speedup vs baseline: 1.0054x; 1.0054x over previous
"""Bahdanau additive attention on 8 Trainium2 NeuronCores.

Data-parallel: batch dim B=8 sharded 1 batch per core; weights replicated.

Per-core math (Te=256 enc steps, Td=128 dec steps, H=512):
  enc_w = enc_out @ W            [Te, H]
  dec_u = dec_out @ U            [Td, H]
  scores[d,t] = sum_k v[k] * tanh(enc_w[t,k] + dec_u[d,k])
  e = softmax(scores, axis=t)    [Td, Te]
  c = e @ enc_out                [Td, H]

Layout: k (=H) lives on SBUF partitions (4 chunks of 128). The add
broadcasts dec_u^T[:, d] as a per-partition scalar on DVE; tanh runs as
large-free-dim ACT instructions; the v-weighted k-reduction runs on PE
with a replicated-column stationary (v (x) ones_32) col-tiled so that
decoder step d = 32*j + g lands on PSUM partition d directly.
"""

import os
import numpy as np

B, Te, Td, H = 8, 256, 128, 512
N_CORES = 8
KC = H // 128  # 4 k-chunks

# score-path dtype: "f32" or "bf16"
SCORE_DTYPE = os.environ.get("BAHDANAU_SCORE_DTYPE", "f32")

_BUILT = {}


def _build():
    key = SCORE_DTYPE
    if key in _BUILT:
        return _BUILT[key]

    import concourse.bass as bass
    import concourse.tile as tile
    from concourse import bacc, mybir

    dt = mybir.dt
    f32 = dt.float32
    f32r = dt.float32r
    # score-path storage dtype for the broadcast-add sums (ACT tanh input).
    # f32r = fp32 bits, PE processes at 1 cycle/row (vs 4 for strict fp32).
    sdt = dt.float32r if SCORE_DTYPE == "f32" else dt.bfloat16
    # tanh OUTPUT / v-reduce dtype: walrus rejects f32r matmuls with
    # tile_position (col-tiling), so the v-weighted reduce runs in fp16
    # (tanh outputs live in [-1,1] -> fp16's 10-bit mantissa beats bf16 4x);
    # the tanh args themselves stay exact fp32.
    tdt = dt.float16

    nc = bacc.Bacc("TRN2", target_bir_lowering=False, debug=False)

    enc_d = nc.dram_tensor("enc", [Te, H], f32r, kind="ExternalInput")
    encT_d = nc.dram_tensor("encT", [H, Te], f32r, kind="ExternalInput")
    decT_d = nc.dram_tensor("decT", [H, Td], f32r, kind="ExternalInput")
    W_d = nc.dram_tensor("W", [H, H], f32r, kind="ExternalInput")
    U_d = nc.dram_tensor("U", [H, H], f32r, kind="ExternalInput")
    vrep_d = nc.dram_tensor("vrep", [H, 32], tdt, kind="ExternalInput")
    masks_d = nc.dram_tensor("masks", [32, 128, Te], dt.uint8, kind="ExternalInput")
    ident_d = nc.dram_tensor("ident", [128, 128], f32, kind="ExternalInput")
    c_d = nc.dram_tensor("c_out", [Td, H], f32, kind="ExternalOutput")
    e_d = nc.dram_tensor("e_out", [Td, Te], f32, kind="ExternalOutput")

    X = mybir.AxisListType.X
    Tanh = mybir.ActivationFunctionType.Tanh
    Exp = mybir.ActivationFunctionType.Exp

    with tile.TileContext(nc) as tc:
        with (
            tc.tile_pool(name="const", bufs=1) as const,
            tc.tile_pool(name="ew", bufs=1) as ew,
            tc.tile_pool(name="sums", bufs=4) as sums,
            tc.tile_pool(name="tanhp", bufs=4) as tanhp,
            tc.tile_pool(name="sm", bufs=1) as sm,
            tc.tile_pool(name="vred", bufs=4, space=bass.MemorySpace.PSUM) as vred,
            tc.tile_pool(name="misc_ps", bufs=2, space=bass.MemorySpace.PSUM) as misc_ps,
        ):
            # ---- load inputs ----
            # W/U are loaded per 128-column block so the first enc_wT /
            # dec_uT matmuls unblock as early as possible.
            # DMA order follows the dependency chain of the first tanh:
            # the enc production chain needs only encT + W block 0, so those
            # go first; decT/U block 0 follow, then the remaining blocks.
            encT_sb = const.tile([128, KC, Te], f32r, tag="encT_sb")
            nc.sync.dma_start(encT_sb[:], encT_d.ap().rearrange("(c p) t -> p c t", p=128))
            W_sb = const.tile([128, KC, H], f32r, tag="W_sb")
            W_re = W_d.ap().rearrange("(c p) k -> p c k", p=128)
            nc.sync.dma_start(W_sb[:, :, 0:128], W_re[:, :, 0:128])
            decT_sb = const.tile([128, KC, Td], f32r, tag="decT_sb")
            nc.sync.dma_start(decT_sb[:], decT_d.ap().rearrange("(c p) d -> p c d", p=128))
            U_sb = const.tile([128, KC, H], f32r, tag="U_sb")
            U_re = U_d.ap().rearrange("(c p) k -> p c k", p=128)
            nc.sync.dma_start(U_sb[:, :, 0:128], U_re[:, :, 0:128])
            for m in range(1, KC):
                nc.sync.dma_start(
                    W_sb[:, :, m * 128:(m + 1) * 128], W_re[:, :, m * 128:(m + 1) * 128]
                )
                nc.sync.dma_start(
                    U_sb[:, :, m * 128:(m + 1) * 128], U_re[:, :, m * 128:(m + 1) * 128]
                )
            vrep_sb = const.tile([128, KC, 32], tdt, tag="vrep_sb")
            nc.sync.dma_start(vrep_sb[:], vrep_d.ap().rearrange("(c p) m -> p c m", p=128))
            masks_sb = const.tile([128, 32, Te], dt.uint8, tag="masks_sb")
            nc.sync.dma_start(masks_sb[:], masks_d.ap().rearrange("g p t -> p g t"))

            # ---- enc_wT[k, t] = sum_h' W[h', k] * encT[h', t]  (4 m-chunks),
            # interleaved with dec_uT so chunk m of both is ready early ----
            enc_wT = ew.tile([128, KC, Te], sdt, tag="enc_wT")
            dec_uT = ew.tile([128, KC, Td], f32, tag="dec_uT")
            for m in range(KC):
                ps = misc_ps.tile([128, H], f32, tag="misc")
                for kk in range(KC):
                    nc.tensor.matmul(
                        ps[:, :Te],
                        W_sb[:, kk, m * 128:(m + 1) * 128],
                        encT_sb[:, kk, :],
                        start=(kk == 0),
                        stop=(kk == KC - 1),
                    )
                nc.scalar.copy(enc_wT[:, m, :], ps[:, :Te])
                ps = misc_ps.tile([128, H], f32, tag="misc")
                for kk in range(KC):
                    nc.tensor.matmul(
                        ps[:, :Td],
                        U_sb[:, kk, m * 128:(m + 1) * 128],
                        decT_sb[:, kk, :],
                        start=(kk == 0),
                        stop=(kk == KC - 1),
                    )
                nc.scalar.copy(dec_uT[:, m, :], ps[:, :Td])

            # ---- scores via per-quad pipeline ----
            # quad g handles decoder steps {g, 32+g, 64+g, 96+g}
            scores = sm.tile([128, Te], f32, tag="scores")
            for g in range(32):
                S = sums.tile([128, 16, Te], sdt, tag="S")
                for kc in range(KC):
                    for j in range(4):
                        d = 32 * j + g
                        nc.vector.tensor_scalar_add(
                            S[:, kc * 4 + j, :],
                            enc_wT[:, kc, :],
                            dec_uT[:, kc, d:d + 1],
                        )
                T = tanhp.tile([128, 16, Te], tdt, tag="T")
                edge = g < 2 or g == 31
                if edge:
                    # fine-grained tanh at the edges: at the start ACT can
                    # begin once enc_wT/dec_uT chunk 0 exists; at the end the
                    # PE reduce overlaps the tanh tail instead of serializing
                    for kc in range(KC):
                        s0 = kc * 4
                        nc.scalar.activation(
                            T[:, s0:s0 + 4, :], S[:, s0:s0 + 4, :], Tanh
                        )
                else:
                    nc.scalar.activation(T[:], S[:], Tanh)
                P = vred.tile([128, Te], f32, tag="P")
                # for the last quad run the accumulation kc-major so the PE
                # starts on chunk 0 while chunk 1..3 tanh still runs (the four
                # per-col-group accumulation chains interleave; has_written
                # is per element so this is safe on HW)
                loop = (
                    [(j, kc) for kc in range(KC) for j in range(4)]
                    if g == 31
                    else [(j, kc) for j in range(4) for kc in range(KC)]
                )
                for j, kc in loop:
                    nc.tensor.matmul(
                        P[32 * j:32 * (j + 1), :],
                        vrep_sb[:, kc, :],
                        T[:, kc * 4 + j, :],
                        start=(kc == 0),
                        stop=(kc == KC - 1),
                        tile_position=(0, 32 * j),
                        skip_group_check=(g == 31),
                    )
                nc.vector.copy_predicated(scores[:], masks_sb[:, g, :], P[:])

            # inputs needed only by the tail, loaded at low priority
            enc_sb = const.tile([128, 2, H], f32r, tag="enc_sb")
            nc.sync.dma_start(enc_sb[:], enc_d.ap().rearrange("(c p) h -> p c h", p=128))
            ident_sb = const.tile([128, 128], f32, tag="ident_sb")
            nc.sync.dma_start(ident_sb[:], ident_d.ap())

            # ---- softmax over t (free dim) + context c = e @ enc_out ----
            # context uses the UNNORMALIZED exp u and folds 1/Z into the
            # final PSUM->SBUF copy, so the e^T transpose and context matmul
            # don't wait on the row-sum/reciprocal.
            u = sm.tile([128, Te], f32, tag="u")
            nc.scalar.activation(u[:], scores[:], Exp)
            uT = sm.tile([128, 2, 128], f32r, tag="uT")
            for tc_i in range(2):
                ps = misc_ps.tile([128, H], f32, tag="misc")
                nc.tensor.transpose(
                    ps[:, :128], u[:, tc_i * 128:(tc_i + 1) * 128], ident_sb[:]
                )
                nc.scalar.copy(uT[:, tc_i, :], ps[:, :128])
            z = sm.tile([128, 1], f32, tag="z")
            nc.vector.tensor_reduce(z[:], u[:], axis=X, op=mybir.AluOpType.add)
            rz = sm.tile([128, 1], f32, tag="rz")
            nc.vector.reciprocal(rz[:], z[:])
            e_sb = sm.tile([128, Te], f32, tag="e_sb")
            nc.vector.tensor_scalar_mul(e_sb[:], u[:], rz[:, 0:1])
            nc.sync.dma_start(e_d.ap(), e_sb[:])
            cps = misc_ps.tile([128, H], f32, tag="misc")
            for tc_i in range(2):
                nc.tensor.matmul(
                    cps[:],
                    uT[:, tc_i, :],
                    enc_sb[:, tc_i, :],
                    start=(tc_i == 0),
                    stop=(tc_i == 1),
                )
            c_sb = sm.tile([128, H], f32, tag="c_sb")
            for h in range(2):
                nc.scalar.activation(
                    c_sb[:, h * 256:(h + 1) * 256],
                    cps[:, h * 256:(h + 1) * 256],
                    mybir.ActivationFunctionType.Copy,
                    scale=rz[:, 0:1],
                )
                nc.sync.dma_start(
                    c_d.ap()[:, h * 256:(h + 1) * 256], c_sb[:, h * 256:(h + 1) * 256]
                )

    nc.compile()
    _BUILT[key] = nc
    return nc


def _make_in_maps(enc_out, dec_out, W, U, v):
    sdt_np = np.float16
    enc_out = np.asarray(enc_out, dtype=np.float32)
    dec_out = np.asarray(dec_out, dtype=np.float32)
    W = np.ascontiguousarray(np.asarray(W, dtype=np.float32))
    U = np.ascontiguousarray(np.asarray(U, dtype=np.float32))
    v = np.asarray(v, dtype=np.float32)
    vrep = np.ascontiguousarray(np.repeat(v.reshape(H, 1), 32, axis=1)).astype(sdt_np)
    ident = np.eye(128, dtype=np.float32)
    masks = np.zeros((32, 128, Te), np.uint8)
    for g in range(32):
        masks[g, g::32, :] = 1
    in_maps = []
    for b in range(B):
        in_maps.append(
            {
                "enc": np.ascontiguousarray(enc_out[b]),
                "encT": np.ascontiguousarray(enc_out[b].T),
                "decT": np.ascontiguousarray(dec_out[b].T),
                "W": W,
                "U": U,
                "vrep": vrep,
                "masks": masks,
                "ident": ident,
            }
        )
    return in_maps


def run(enc_out, dec_out, W, U, v, trace=False):
    """Build+run on 8 cores. Returns (c, e, BassKernelResults)."""
    from concourse.bass_utils import run_bass_kernel_spmd

    nc = _build()
    in_maps = _make_in_maps(enc_out, dec_out, W, U, v)
    res = run_bass_kernel_spmd(nc, in_maps, list(range(N_CORES)), trace=trace)
    rs = res.results
    c = np.stack([np.asarray(rs[i]["c_out"]) for i in range(N_CORES)])
    e = np.stack([np.asarray(rs[i]["e_out"]) for i in range(N_CORES)])
    return c.astype(np.float32), e.astype(np.float32), res


def kernel(enc_out, dec_out, W, U, v):
    c, e, _ = run(enc_out, dec_out, W, U, v)
    return c, e
